# revision 16
# baseline (speedup 1.0000x reference)
"""Trainium2 Bass kernel for CausalModulatedAttention.

Full-input contract: kernel(**inputs) takes the unsharded numpy inputs and
returns the full (B, L, D) float32 output.

Sharding: core = 2*b + g (b = batch, g = head-group).  The two cores of a
batch split the 16 heads (8 each) but both cover all 512 rows, with TRUE
causal extents per 128-row chunk (jext = 128*(ic+1)) -- no wasted score
columns.  The pairwise causal-graph bias G (shared by all heads) is instead
row-sharded across the pair -- core g computes G rows {0,3} or {1,2} chunks
(widths 256/512, SPMD-uniform) -- and the 192KB bias tiles are exchanged
through a per-pair AllGather.  Each core produces a partial output (its
heads' half of the d-contraction in the final projection); the host adds
the two halves.

Per core:
  - k/q/v projections in transposed layouts straight from x^T (PE),
    head-group slices only
  - pairwise G: gelu(he[j,c]+hc[i,c]+b1[c]) as one ACT op per 4-row group
    (per-partition bias), reduced over c via per-t stationary matrices (PE)
  - scores = q.k^T (PE); bias+mask tile added into PSUM on DVE
  - softmax without max-subtraction; Exp emits row sums via accum_out
  - transpose+normalize fused: EnT = E_chunk^T @ diag(1/rowsum) on PE
  - attn @ v on PE (col-group packed head pairs); partial output proj (PE)
All matmul operands bf16, fp32 PSUM accumulation.
"""

import math

import numpy as np
import ml_dtypes

import concourse.bass as bass
import concourse.mybir as mybir
import concourse.tile as tile
from concourse import bacc
from concourse.bass_utils import run_bass_kernel_spmd

BF = mybir.dt.bfloat16
F32 = mybir.dt.float32
AF = mybir.ActivationFunctionType
ALU = mybir.AluOpType

B, L, D = 4, 512, 1024
H, HD, CD = 16, 64, 32
ALPHA = 0.3
N_CORES = 8
HPC = 8               # heads per core
DPC = HPC * HD        # 512 d-columns per core
GJX = [256, 512]      # pairwise G width for (lo, hi) owned row chunk
NEG = -1.0e30
GW = GJX[0] + GJX[1]  # 768: packed G width per core
# packed bf16 consts: w2t (4096) | mask (768) | ident (128)
CPK = 4096 + GW + 128


def _bf(a):
    return np.ascontiguousarray(a.astype(ml_dtypes.bfloat16))


def _f32(a):
    return np.ascontiguousarray(a.astype(np.float32))


def core_rows(g):
    """Global row ranges (lo, hi) whose G rows core-group g computes."""
    lo = range(g * 128, g * 128 + 128)
    hi = range(384 - g * 128, 384 - g * 128 + 128)
    return lo, hi


def build_program():
    nc = bacc.Bacc("TRN2", num_devices=N_CORES, target_bir_lowering=False,
                   debug=False)

    boot_d = nc.dram_tensor("boot", [128, 512], BF, kind="ExternalInput")
    xall_d = nc.dram_tensor("xall", [128, 8 * L + 8 * 256], BF, kind="ExternalInput")
    cpk_d = nc.dram_tensor("cpk", [128, CPK], BF, kind="ExternalInput")
    wk_d = nc.dram_tensor("wka", [128, 8 * DPC], BF, kind="ExternalInput")
    wq_d = nc.dram_tensor("wqa", [128, 8 * DPC], BF, kind="ExternalInput")
    wv_d = nc.dram_tensor("wva", [128, 8 * DPC], BF, kind="ExternalInput")
    wo_d = nc.dram_tensor("woa", [128, 4 * D], BF, kind="ExternalInput")
    b1_d = nc.dram_tensor("b1c", [CD, 1], F32, kind="ExternalInput")
    b2_d = nc.dram_tensor("b2h", [128, 1], F32, kind="ExternalInput")
    out_d = nc.dram_tensor("out", [L, D], F32, kind="ExternalOutput")

    with tile.TileContext(nc) as tc:
        with (
            tc.tile_pool(name="consts", bufs=1) as consts,
            tc.tile_pool(name="work", bufs=3) as work,
            tc.tile_pool(name="entp", bufs=6) as entp,
            tc.tile_pool(name="dram", bufs=1, space="DRAM") as dpool,
            tc.tile_pool(name="ppbig", bufs=3, space="PSUM") as ppbig,
            tc.tile_pool(name="ppg", bufs=1, space="PSUM") as ppg,
            tc.tile_pool(name="pptp", bufs=2, space="PSUM") as pptp,
            tc.tile_pool(name="ppot", bufs=2, space="PSUM") as ppot,
        ):
            def load(name, shape, dt, src):
                t = consts.tile(shape, dt, tag=name)
                nc.sync.dma_start(out=t[:], in_=src)
                return t

            boot = load("boot", [128, 512], BF, boot_d[:, :])
            xall = load("xall", [128, 8 * L + 8 * 256], BF, xall_d[:, :])
            cpk = load("cpk", [128, CPK], BF, cpk_d[:, :])
            b1c = load("b1c", [CD, 1], F32, b1_d[:, :])
            b2h = load("b2h", [128, 1], F32, b2_d[:, :])
            wka = load("wka", [128, 8 * DPC], BF, wk_d[:, :])
            wqa = load("wqa", [128, 8 * DPC], BF, wq_d[:, :])
            wva = load("wva", [128, 8 * DPC], BF, wv_d[:, :])
            woa = load("woa", [128, 4 * D], BF, wo_d[:, :])

            xT = [xall[:, mc * L:(mc + 1) * L] for mc in range(8)]
            xTq = [xall[:, 8 * L + mc * 256: 8 * L + (mc + 1) * 256]
                   for mc in range(8)]
            wc1 = boot[:, 0:256]
            we1 = boot[:, 256:512]
            w2t = cpk[:, 0:4096]
            maskc = cpk[:, 4096:4096 + GW]
            ident = cpk[:, 4096 + GW:4096 + GW + 128]
            wk = [wka[:, mc * DPC:(mc + 1) * DPC] for mc in range(8)]
            wq = [wqa[:, mc * DPC:(mc + 1) * DPC] for mc in range(8)]
            wv = [wva[:, mc * DPC:(mc + 1) * DPC] for mc in range(8)]
            wo = [woa[:, dc * D:(dc + 1) * D] for dc in range(4)]

            # ---------- he / hc (unblock the gelu chain) ----------
            ps = ppbig.tile([CD, L], F32, tag="ps")
            for mc in range(8):
                nc.tensor.matmul(ps[:], we1[:, mc * CD:(mc + 1) * CD], xT[mc],
                                 start=(mc == 0), stop=(mc == 7))
            he4 = consts.tile([128, L], BF, tag="he4")
            nc.vector.tensor_copy(he4[0:CD, :], ps[:])
            for u in range(1, 4):
                nc.sync.dma_start(out=he4[u * CD:(u + 1) * CD, :], in_=he4[0:CD, :])

            ps = ppbig.tile([CD, 256], F32, tag="ps")
            for mc in range(8):
                nc.tensor.matmul(ps[:], wc1[:, mc * CD:(mc + 1) * CD], xTq[mc],
                                 start=(mc == 0), stop=(mc == 7))
            hcbT = consts.tile([CD, 256], F32, tag="hcbT")
            nc.vector.tensor_scalar_add(hcbT[:], ps[:], b1c[:, 0:1])
            hc4 = consts.tile([128, 64], F32, tag="hc4")
            hsrc = hcbT[:, :].rearrange("p (a t f) -> p a t f", a=2, t=32)
            for u in range(4):
                nc.sync.dma_start(
                    out=hc4[u * CD:(u + 1) * CD, :].rearrange("p (a t) -> p a t", a=2),
                    in_=hsrc[:, :, :, u])

            # ---------- pairwise causal-graph bias (owned rows) ----------
            gsend = consts.tile([128, GW], BF, tag="gsend")

            def pairwise(oc):           # oc: owned chunk 0 (lo) / 1 (hi)
                jx = GJX[oc]
                moff = 0 if oc == 0 else GJX[0]
                graw = ppg.tile([128, 512], F32, tag="graw")
                for t in range(32):
                    ga = work.tile([128, jx], BF, tag=f"ga{oc}")
                    nc.scalar.activation(ga[:], he4[:, :jx], AF.Gelu,
                                         bias=hc4[:, oc * 32 + t: oc * 32 + t + 1])
                    nc.tensor.matmul(graw[:, :jx], w2t[:, t * 128:(t + 1) * 128],
                                     ga[:], start=(t == 0), stop=(t == 31))
                th = work.tile([128, jx], BF, tag=f"th{oc}")
                nc.scalar.activation(th[:], graw[:, :jx], AF.Tanh, scale=0.5,
                                     bias=b2h[:, 0:1])
                nc.vector.scalar_tensor_tensor(
                    gsend[:, moff:moff + jx], th[:], ALPHA / 2.0,
                    maskc[:, moff:moff + jx], op0=ALU.mult, op1=ALU.add)

            pairwise(0)
            pairwise(1)

            # ---------- exchange G-bias within the batch pair ----------
            gin = dpool.tile([128, GW], BF, tag="gin")
            gout = dpool.tile([2, 128, GW], BF, tag="gout")
            nc.sync.dma_start(out=gin[:], in_=gsend[:])
            nc.gpsimd.collective_compute(
                "AllGather", ALU.bypass,
                replica_groups=[[0, 1], [2, 3], [4, 5], [6, 7]],
                ins=[gin[:, :].opt()], outs=[gout[:, :, :].opt()])
            gb0 = consts.tile([128, GW], BF, tag="gb0")
            gb1 = consts.tile([128, GW], BF, tag="gb1")
            nc.sync.dma_start(out=gb0[:], in_=gout[0, :, :])
            nc.sync.dma_start(out=gb1[:], in_=gout[1, :, :])
            # global row-chunk ic -> (bias tile, column offset); G rows
            # {0,3} came from rank 0, {1,2} from rank 1.
            gmap = {0: (gb0, 0), 3: (gb0, GJX[0]),
                    1: (gb1, 0), 2: (gb1, GJX[0])}

            # ---------- projection emitters ----------
            kT, qT, v = [None] * 4, [None] * 4, [None] * 4

            def proj_kq(dc):
                ps = ppbig.tile([128, L], F32, tag="ps")
                for mc in range(8):
                    nc.tensor.matmul(ps[:], wk[mc][:, dc * 128:(dc + 1) * 128],
                                     xT[mc], start=(mc == 0), stop=(mc == 7))
                t = consts.tile([128, L], BF, tag=f"kT{dc}")
                nc.vector.tensor_copy(t[:], ps[:])
                kT[dc] = t
                ps = ppbig.tile([128, L], F32, tag="ps")
                for mc in range(8):
                    nc.tensor.matmul(ps[:], wq[mc][:, dc * 128:(dc + 1) * 128],
                                     xT[mc], start=(mc == 0), stop=(mc == 7))
                t = consts.tile([128, L], BF, tag=f"qT{dc}")
                nc.vector.tensor_copy(t[:], ps[:])
                qT[dc] = t

            def proj_v(jc):
                t = consts.tile([128, DPC], BF, tag=f"v{jc}")
                ps = ppbig.tile([128, DPC], F32, tag="ps")
                for mc in range(8):
                    nc.tensor.matmul(ps[:], xT[mc][:, jc * 128:(jc + 1) * 128],
                                     wv[mc], start=(mc == 0), stop=(mc == 7))
                nc.vector.tensor_copy(t[:], ps[:])
                v[jc] = t

            # ---------- attention ----------
            ot = [[None] * 4 for _ in range(4)]

            def attention(ic, hp):
                jx = 128 * (ic + 1)
                njc = ic + 1
                gt, go = gmap[ic]
                otp = ppot.tile([128, 128], F32, tag="otp")
                for sub in range(2):
                    h = 2 * hp + sub
                    po = 64 * sub
                    sc = ppbig.tile([128, 512], F32, tag="ps")
                    nc.tensor.matmul(
                        sc[:, :jx], qT[hp][po:po + 64, ic * 128:(ic + 1) * 128],
                        kT[hp][po:po + 64, :jx], start=True, stop=True,
                        tile_position=(po, 0))
                    nc.vector.tensor_add(sc[:, :jx], sc[:, :jx],
                                         gt[:, go:go + jx])
                    e = work.tile([128, jx], BF, tag=f"e{ic}")
                    sums = work.tile([128, 1], F32, tag="sums")
                    nc.scalar.activation(e[:], sc[:, :jx], AF.Exp,
                                         accum_out=sums[:, 0:1])
                    inv = work.tile([128, 1], F32, tag="inv")
                    nc.vector.reciprocal(inv[:], sums[:])
                    dg = work.tile([128, 128], BF, tag="dg")
                    nc.vector.tensor_scalar_mul(dg[:], ident, inv[:, 0:1])
                    for jc in range(njc):
                        etp = pptp.tile([128, 128], F32, tag="etp")
                        nc.tensor.matmul(etp[:], e[:, jc * 128:(jc + 1) * 128],
                                         dg[:], start=True, stop=True)
                        ent = entp.tile([128, 128], BF, tag="ent")
                        if jc % 2 == 0:
                            nc.vector.tensor_copy(ent[:], etp[:])
                        else:
                            nc.scalar.copy(ent[:], etp[:])
                        nc.tensor.matmul(
                            otp[po:po + 64, :], v[jc][:, h * HD:(h + 1) * HD],
                            ent[:], start=(jc == 0), stop=(jc == njc - 1),
                            tile_position=(0, po))
                t = consts.tile([128, 128], BF, tag=f"ot{ic}_{hp}")
                nc.vector.tensor_copy(t[:], otp[:])
                ot[ic][hp] = t

            def out_proj(ic, nn):
                ps = ppbig.tile([128, 512], F32, tag="ps")
                for dc in range(4):
                    nc.tensor.matmul(ps[:], ot[ic][dc][:],
                                     wo[dc][:, nn * 512:(nn + 1) * 512],
                                     start=(dc == 0), stop=(dc == 3))
                osb = work.tile([128, 512], F32, tag="osb")
                nc.vector.tensor_copy(osb[:], ps[:])
                nc.sync.dma_start(
                    out=out_d[ic * 128:(ic + 1) * 128, nn * 512:(nn + 1) * 512],
                    in_=osb[:])

            # emission: projections, then attention largest-first with the
            # finished chunks' output projections interleaved as PE fillers
            for dc in range(4):
                proj_kq(dc)
                proj_v(dc)
            for hp in range(4):
                attention(3, hp)
            for hp in range(4):
                attention(2, hp)
                if hp >= 2:
                    out_proj(3, hp - 2)
            for hp in range(4):
                attention(1, hp)
                if hp >= 2:
                    out_proj(2, hp - 2)
            for hp in range(4):
                attention(0, hp)
                if hp >= 2:
                    out_proj(1, hp - 2)
            out_proj(0, 0)
            out_proj(0, 1)

    nc.compile()
    return nc


def _host_inputs(x, Wq, Wk, Wv, Wo, Wc, We, W1c, W1e, b1, W2, b2):
    """Per-core input dicts (host-side shard/cast/pack)."""
    x = _f32(np.asarray(x))
    wq_s = _f32(np.asarray(Wq) / math.sqrt(HD))
    wk = _f32(np.asarray(Wk))
    wv = _f32(np.asarray(Wv))
    wo = _f32(np.asarray(Wo))
    wc1 = _f32(np.asarray(Wc) @ np.asarray(W1c))      # (D, CD)
    we1 = _f32(np.asarray(We) @ np.asarray(W1e))
    wc1r = wc1.reshape(8, 128, CD).transpose(1, 0, 2).reshape(128, 8 * CD)
    we1r = we1.reshape(8, 128, CD).transpose(1, 0, 2).reshape(128, 8 * CD)
    b1c = _f32(np.asarray(b1).reshape(CD, 1))
    b2h = _f32(np.full((128, 1), 0.5 * float(np.asarray(b2).reshape(-1)[0])))
    w2 = _f32(np.asarray(W2))

    # w2t[p=u*32+c, t*128 + m] = W2[c] if m == 4t+u else 0
    w2t = np.zeros((32, 128, 128), np.float32)
    for t in range(32):
        for u in range(4):
            w2t[t, u * CD:(u + 1) * CD, 4 * t + u] = w2
    w2t = w2t.transpose(1, 0, 2).reshape(128, 32 * 128)

    identb = np.eye(128, dtype=np.float32)
    bootc = np.concatenate([wc1r, we1r], axis=1)

    def hpack(w, cols):  # (1024, cols) -> (128, 8*cols) m-chunk-major
        return w.reshape(8, 128, cols).transpose(1, 0, 2).reshape(128, 8 * cols)

    in_maps = []
    for core in range(N_CORES):
        b, g = core // 2, core % 2
        lo, hi = core_rows(g)
        rows = np.concatenate([np.arange(lo.start, lo.stop),
                               np.arange(hi.start, hi.stop)])
        hd0 = g * DPC                                  # head-group d offset
        xTb = np.ascontiguousarray(x[b].T)             # (D, L)
        mask = np.zeros((128, GW), np.float32)
        moff = 0
        for oc, rng in enumerate((lo, hi)):
            jx = GJX[oc]
            jj = np.arange(jx)[None, :]
            rr = np.arange(rng.start, rng.stop)[:, None]
            mask[:, moff:moff + jx] = np.where(jj <= rr, 0.0, NEG)
            moff += jx
        xTb8 = hpack(xTb, L)
        xTq8 = hpack(np.ascontiguousarray(xTb[:, rows]), 256)
        in_maps.append({
            "boot": _bf(bootc),
            "xall": _bf(np.concatenate([xTb8, xTq8], axis=1)),
            "cpk": _bf(np.concatenate([w2t, mask, identb], axis=1)),
            "wka": _bf(hpack(wk[:, hd0:hd0 + DPC], DPC)),
            "wqa": _bf(hpack(wq_s[:, hd0:hd0 + DPC], DPC)),
            "wva": _bf(hpack(wv[:, hd0:hd0 + DPC], DPC)),
            "woa": _bf(np.ascontiguousarray(
                wo[hd0:hd0 + DPC].reshape(4, 128, D)
                .transpose(1, 0, 2).reshape(128, 4 * D))),
            "b1c": b1c, "b2h": b2h,
        })
    return in_maps


def run(inputs: dict, trace: bool = False):
    """Build, run on 8 cores, return (full_output, BassKernelResults)."""
    nc = build_program()
    in_maps = _host_inputs(**inputs)
    res = run_bass_kernel_spmd(nc, in_maps, core_ids=list(range(N_CORES)),
                               trace=trace)
    out = np.zeros((B, L, D), np.float32)
    for b in range(B):
        out[b] = res.results[2 * b]["out"] + res.results[2 * b + 1]["out"]
    return out, res


def kernel(**inputs) -> np.ndarray:
    out, _ = run(inputs, trace=False)
    return out


# revision 21
# speedup vs baseline: 1.1377x; 1.1377x over previous
"""Trainium2 Bass kernel for CausalModulatedAttention.

Full-input contract: kernel(**inputs) takes the unsharded numpy inputs and
returns the full (B, L, D) float32 output.

Sharding: core = 2*b + g (b = batch, g = head-group).  The two cores of a
batch split the 16 heads (8 each) but both cover all 512 rows, with TRUE
causal extents per 128-row chunk (jext = 128*(ic+1)) -- no wasted score
columns.  The pairwise causal-graph bias G (shared by all heads) is instead
row-sharded across the pair -- core g computes G rows {0,3} or {1,2} chunks
(widths 256/512, SPMD-uniform) -- and the 192KB bias tiles are exchanged
through a per-pair AllGather.  Each core produces a partial output (its
heads' half of the d-contraction in the final projection); the host adds
the two halves.

Per core:
  - k/q/v projections in transposed layouts straight from x^T (PE),
    head-group slices only
  - pairwise G: gelu(he[j,c]+hc[i,c]+b1[c]) as one ACT op per 4-row group
    (per-partition bias), reduced over c via per-t stationary matrices (PE)
  - scores = q.k^T (PE); bias+mask tile added into PSUM on DVE
  - softmax without max-subtraction; Exp emits row sums via accum_out
  - transpose+normalize fused: EnT = E_chunk^T @ diag(1/rowsum) on PE
  - attn @ v on PE (col-group packed head pairs); partial output proj (PE)
All matmul operands bf16, fp32 PSUM accumulation.
"""

import math

import numpy as np
import ml_dtypes

import concourse.bass as bass
import concourse.mybir as mybir
import concourse.tile as tile
from concourse import bacc
from concourse.bass_utils import run_bass_kernel_spmd

BF = mybir.dt.bfloat16
F32 = mybir.dt.float32
AF = mybir.ActivationFunctionType
ALU = mybir.AluOpType

B, L, D = 4, 512, 1024
H, HD, CD = 16, 64, 32
ALPHA = 0.3
N_CORES = 8
HPC = 8               # heads per core
DPC = HPC * HD        # 512 d-columns per core
GJX = [256, 512]      # pairwise G width for (lo, hi) owned row chunk
NEG = -1.0e30
GW = GJX[0] + GJX[1]  # 768: packed G width per core
# packed bf16 consts: w2t (4096) | mask (768) | ident (128)
CPK = 4096 + GW + 128


def _bf(a):
    return np.ascontiguousarray(a.astype(ml_dtypes.bfloat16))


def _f32(a):
    return np.ascontiguousarray(a.astype(np.float32))


def core_rows(g):
    """Global row ranges (lo, hi) whose G rows core-group g computes."""
    lo = range(g * 128, g * 128 + 128)
    hi = range(384 - g * 128, 384 - g * 128 + 128)
    return lo, hi


def build_program():
    nc = bacc.Bacc("TRN2", num_devices=N_CORES, target_bir_lowering=False,
                   debug=False)

    boot_d = nc.dram_tensor("boot", [128, 512], BF, kind="ExternalInput")
    xall_d = nc.dram_tensor("xall", [128, 8 * L + 8 * 256], BF, kind="ExternalInput")
    cpk_d = nc.dram_tensor("cpk", [128, CPK], BF, kind="ExternalInput")
    wk_d = nc.dram_tensor("wka", [128, 8 * DPC], BF, kind="ExternalInput")
    wq_d = nc.dram_tensor("wqa", [128, 8 * DPC], BF, kind="ExternalInput")
    wv_d = nc.dram_tensor("wva", [128, 8 * DPC], BF, kind="ExternalInput")
    wo_d = nc.dram_tensor("woa", [128, 4 * D], BF, kind="ExternalInput")
    b1_d = nc.dram_tensor("b1c", [CD, 1], F32, kind="ExternalInput")
    b2_d = nc.dram_tensor("b2h", [128, 1], F32, kind="ExternalInput")
    out_d = nc.dram_tensor("out", [L, D], F32, kind="ExternalOutput")

    with tile.TileContext(nc) as tc:
        with (
            tc.tile_pool(name="consts", bufs=1) as consts,
            tc.tile_pool(name="work", bufs=3) as work,
            tc.tile_pool(name="entp", bufs=6) as entp,
            tc.tile_pool(name="dram", bufs=1, space="DRAM") as dpool,
            tc.tile_pool(name="ppbig", bufs=3, space="PSUM") as ppbig,
            tc.tile_pool(name="ppg", bufs=1, space="PSUM") as ppg,
            tc.tile_pool(name="pptp", bufs=2, space="PSUM") as pptp,
            tc.tile_pool(name="ppot", bufs=2, space="PSUM") as ppot,
        ):
            def load(name, shape, dt, src):
                t = consts.tile(shape, dt, tag=name)
                nc.sync.dma_start(out=t[:], in_=src)
                return t

            boot = load("boot", [128, 512], BF, boot_d[:, :])
            xall = load("xall", [128, 8 * L + 8 * 256], BF, xall_d[:, :])
            cpk = load("cpk", [128, CPK], BF, cpk_d[:, :])
            b1c = load("b1c", [CD, 1], F32, b1_d[:, :])
            b2h = load("b2h", [128, 1], F32, b2_d[:, :])
            wka = load("wka", [128, 8 * DPC], BF, wk_d[:, :])
            wqa = load("wqa", [128, 8 * DPC], BF, wq_d[:, :])
            wva = load("wva", [128, 8 * DPC], BF, wv_d[:, :])
            woa = load("woa", [128, 4 * D], BF, wo_d[:, :])

            xT = [xall[:, mc * L:(mc + 1) * L] for mc in range(8)]
            xTq = [xall[:, 8 * L + mc * 256: 8 * L + (mc + 1) * 256]
                   for mc in range(8)]
            wc1 = boot[:, 0:256]
            we1 = boot[:, 256:512]
            w2t = cpk[:, 0:4096]
            maskc = cpk[:, 4096:4096 + GW]
            ident = cpk[:, 4096 + GW:4096 + GW + 128]
            wk = [wka[:, mc * DPC:(mc + 1) * DPC] for mc in range(8)]
            wq = [wqa[:, mc * DPC:(mc + 1) * DPC] for mc in range(8)]
            wv = [wva[:, mc * DPC:(mc + 1) * DPC] for mc in range(8)]
            wo = [woa[:, dc * D:(dc + 1) * D] for dc in range(4)]

            # ---------- he / hc (unblock the gelu chain) ----------
            ps = ppbig.tile([CD, L], F32, tag="ps")
            for mc in range(8):
                nc.tensor.matmul(ps[:], we1[:, mc * CD:(mc + 1) * CD], xT[mc],
                                 start=(mc == 0), stop=(mc == 7))
            he4 = consts.tile([128, L], BF, tag="he4")
            nc.vector.tensor_copy(he4[0:CD, :], ps[:])
            for u in range(1, 4):
                nc.gpsimd.dma_start(out=he4[u * CD:(u + 1) * CD, :],
                                    in_=he4[0:CD, :])

            ps = ppbig.tile([CD, 256], F32, tag="ps")
            for mc in range(8):
                nc.tensor.matmul(ps[:], wc1[:, mc * CD:(mc + 1) * CD], xTq[mc],
                                 start=(mc == 0), stop=(mc == 7))
            hcbT = consts.tile([CD, 256], F32, tag="hcbT")
            nc.vector.tensor_scalar_add(hcbT[:], ps[:], b1c[:, 0:1])
            hc4 = consts.tile([128, 64], F32, tag="hc4")
            hsrc = hcbT[:, :].rearrange("p (a t f) -> p a t f", a=2, t=32)
            for u in range(4):
                nc.gpsimd.dma_start(
                    out=hc4[u * CD:(u + 1) * CD, :].rearrange("p (a t) -> p a t", a=2),
                    in_=hsrc[:, :, :, u])

            # ---------- pairwise causal-graph bias (owned rows) ----------
            gsend = consts.tile([128, GW], BF, tag="gsend")

            def pairwise(oc):           # oc: owned chunk 0 (lo) / 1 (hi)
                jx = GJX[oc]
                moff = 0 if oc == 0 else GJX[0]
                graw = ppg.tile([128, 512], F32, tag="graw")
                for t in range(32):
                    # true causal width for this 4-row group, rounded up to
                    # cover both row-groups' SPMD-shared shape; columns
                    # beyond fd land under the -inf mask.  t==0 spans the
                    # full width so start=True clears has_written everywhere.
                    fd = jx if t == 0 else min(jx, (jx - 128) + 4 * t + 4)
                    ga = work.tile([128, fd], BF, tag=f"ga{oc}")
                    nc.scalar.activation(ga[:], he4[:, :fd], AF.Gelu,
                                         bias=hc4[:, oc * 32 + t: oc * 32 + t + 1])
                    nc.tensor.matmul(graw[:, :fd], w2t[:, t * 128:(t + 1) * 128],
                                     ga[:], start=(t == 0), stop=(t == 31))
                th = work.tile([128, jx], BF, tag=f"th{oc}")
                nc.scalar.activation(th[:], graw[:, :jx], AF.Tanh, scale=0.5,
                                     bias=b2h[:, 0:1])
                nc.vector.scalar_tensor_tensor(
                    gsend[:, moff:moff + jx], th[:], ALPHA / 2.0,
                    maskc[:, moff:moff + jx], op0=ALU.mult, op1=ALU.add)

            pairwise(0)
            pairwise(1)

            # ---------- exchange G-bias within the batch pair ----------
            gin = dpool.tile([128, GW], BF, tag="gin")
            gout = dpool.tile([2, 128, GW], BF, tag="gout")
            nc.gpsimd.dma_start(out=gin[:], in_=gsend[:])
            nc.gpsimd.collective_compute(
                "AllGather", ALU.bypass,
                replica_groups=[[0, 1], [2, 3], [4, 5], [6, 7]],
                ins=[gin[:, :].opt()], outs=[gout[:, :, :].opt()])
            gb0 = consts.tile([128, GW], BF, tag="gb0")
            gb1 = consts.tile([128, GW], BF, tag="gb1")
            nc.gpsimd.dma_start(out=gb0[:], in_=gout[0, :, :])
            nc.gpsimd.dma_start(out=gb1[:], in_=gout[1, :, :])
            # global row-chunk ic -> (bias tile, column offset); G rows
            # {0,3} came from rank 0, {1,2} from rank 1.
            gmap = {0: (gb0, 0), 3: (gb0, GJX[0]),
                    1: (gb1, 0), 2: (gb1, GJX[0])}

            # ---------- projection emitters ----------
            kT, qT, v = [None] * 4, [None] * 4, [None] * 4

            def proj_kq(dc):
                ps = ppbig.tile([128, L], F32, tag="ps")
                for mc in range(8):
                    nc.tensor.matmul(ps[:], wk[mc][:, dc * 128:(dc + 1) * 128],
                                     xT[mc], start=(mc == 0), stop=(mc == 7))
                t = consts.tile([128, L], BF, tag=f"kT{dc}")
                nc.vector.tensor_copy(t[:], ps[:])
                kT[dc] = t
                ps = ppbig.tile([128, L], F32, tag="ps")
                for mc in range(8):
                    nc.tensor.matmul(ps[:], wq[mc][:, dc * 128:(dc + 1) * 128],
                                     xT[mc], start=(mc == 0), stop=(mc == 7))
                t = consts.tile([128, L], BF, tag=f"qT{dc}")
                nc.vector.tensor_copy(t[:], ps[:])
                qT[dc] = t

            def proj_v(jc):
                t = consts.tile([128, DPC], BF, tag=f"v{jc}")
                ps = ppbig.tile([128, DPC], F32, tag="ps")
                for mc in range(8):
                    nc.tensor.matmul(ps[:], xT[mc][:, jc * 128:(jc + 1) * 128],
                                     wv[mc], start=(mc == 0), stop=(mc == 7))
                nc.vector.tensor_copy(t[:], ps[:])
                v[jc] = t

            # ---------- attention ----------
            ot = [[None] * 4 for _ in range(4)]

            def attention(ic, hp):
                jx = 128 * (ic + 1)
                njc = ic + 1
                gt, go = gmap[ic]
                otp = ppot.tile([128, 128], F32, tag="otp")
                for sub in range(2):
                    h = 2 * hp + sub
                    po = 64 * sub
                    sc = ppbig.tile([128, 512], F32, tag="ps")
                    nc.tensor.matmul(
                        sc[:, :jx], qT[hp][po:po + 64, ic * 128:(ic + 1) * 128],
                        kT[hp][po:po + 64, :jx], start=True, stop=True,
                        tile_position=(po, 0))
                    nc.vector.tensor_add(sc[:, :jx], sc[:, :jx],
                                         gt[:, go:go + jx])
                    e = work.tile([128, jx], BF, tag=f"e{ic}")
                    sums = work.tile([128, 1], F32, tag="sums")
                    nc.scalar.activation(e[:], sc[:, :jx], AF.Exp,
                                         accum_out=sums[:, 0:1])
                    inv = work.tile([128, 1], F32, tag="inv")
                    nc.vector.reciprocal(inv[:], sums[:])
                    dg = work.tile([128, 128], BF, tag="dg")
                    nc.vector.tensor_scalar_mul(dg[:], ident, inv[:, 0:1])
                    for jc in range(njc):
                        etp = pptp.tile([128, 128], F32, tag="etp")
                        nc.tensor.matmul(etp[:], e[:, jc * 128:(jc + 1) * 128],
                                         dg[:], start=True, stop=True)
                        ent = entp.tile([128, 128], BF, tag="ent")
                        if jc % 2 == 0:
                            nc.vector.tensor_copy(ent[:], etp[:])
                        else:
                            nc.scalar.copy(ent[:], etp[:])
                        nc.tensor.matmul(
                            otp[po:po + 64, :], v[jc][:, h * HD:(h + 1) * HD],
                            ent[:], start=(jc == 0), stop=(jc == njc - 1),
                            tile_position=(0, po))
                t = consts.tile([128, 128], BF, tag=f"ot{ic}_{hp}")
                nc.vector.tensor_copy(t[:], otp[:])
                ot[ic][hp] = t

            def out_proj(ic, nn):
                ps = ppbig.tile([128, 512], F32, tag="ps")
                for dc in range(4):
                    nc.tensor.matmul(ps[:], ot[ic][dc][:],
                                     wo[dc][:, nn * 512:(nn + 1) * 512],
                                     start=(dc == 0), stop=(dc == 3))
                osb = work.tile([128, 512], F32, tag="osb")
                nc.vector.tensor_copy(osb[:], ps[:])
                nc.sync.dma_start(
                    out=out_d[ic * 128:(ic + 1) * 128, nn * 512:(nn + 1) * 512],
                    in_=osb[:])

            # emission: projections, then attention largest-first with the
            # finished chunks' output projections interleaved as PE fillers
            for dc in range(4):
                proj_kq(dc)
                proj_v(dc)
            for hp in range(4):
                attention(3, hp)
            for hp in range(4):
                attention(2, hp)
                if hp >= 2:
                    out_proj(3, hp - 2)
            for hp in range(4):
                attention(1, hp)
                if hp >= 2:
                    out_proj(2, hp - 2)
            for hp in range(4):
                attention(0, hp)
                if hp >= 2:
                    out_proj(1, hp - 2)
            out_proj(0, 0)
            out_proj(0, 1)

    nc.compile()
    return nc


def _host_inputs(x, Wq, Wk, Wv, Wo, Wc, We, W1c, W1e, b1, W2, b2):
    """Per-core input dicts (host-side shard/cast/pack)."""
    x = _f32(np.asarray(x))
    wq_s = _f32(np.asarray(Wq) / math.sqrt(HD))
    wk = _f32(np.asarray(Wk))
    wv = _f32(np.asarray(Wv))
    wo = _f32(np.asarray(Wo))
    wc1 = _f32(np.asarray(Wc) @ np.asarray(W1c))      # (D, CD)
    we1 = _f32(np.asarray(We) @ np.asarray(W1e))
    wc1r = wc1.reshape(8, 128, CD).transpose(1, 0, 2).reshape(128, 8 * CD)
    we1r = we1.reshape(8, 128, CD).transpose(1, 0, 2).reshape(128, 8 * CD)
    b1c = _f32(np.asarray(b1).reshape(CD, 1))
    b2h = _f32(np.full((128, 1), 0.5 * float(np.asarray(b2).reshape(-1)[0])))
    w2 = _f32(np.asarray(W2))

    # w2t[p=u*32+c, t*128 + m] = W2[c] if m == 4t+u else 0
    w2t = np.zeros((32, 128, 128), np.float32)
    for t in range(32):
        for u in range(4):
            w2t[t, u * CD:(u + 1) * CD, 4 * t + u] = w2
    w2t = w2t.transpose(1, 0, 2).reshape(128, 32 * 128)

    identb = np.eye(128, dtype=np.float32)
    bootc = np.concatenate([wc1r, we1r], axis=1)

    def hpack(w, cols):  # (1024, cols) -> (128, 8*cols) m-chunk-major
        return w.reshape(8, 128, cols).transpose(1, 0, 2).reshape(128, 8 * cols)

    in_maps = []
    for core in range(N_CORES):
        b, g = core // 2, core % 2
        lo, hi = core_rows(g)
        rows = np.concatenate([np.arange(lo.start, lo.stop),
                               np.arange(hi.start, hi.stop)])
        hd0 = g * DPC                                  # head-group d offset
        xTb = np.ascontiguousarray(x[b].T)             # (D, L)
        mask = np.zeros((128, GW), np.float32)
        moff = 0
        for oc, rng in enumerate((lo, hi)):
            jx = GJX[oc]
            jj = np.arange(jx)[None, :]
            rr = np.arange(rng.start, rng.stop)[:, None]
            mask[:, moff:moff + jx] = np.where(jj <= rr, 0.0, NEG)
            moff += jx
        xTb8 = hpack(xTb, L)
        xTq8 = hpack(np.ascontiguousarray(xTb[:, rows]), 256)
        in_maps.append({
            "boot": _bf(bootc),
            "xall": _bf(np.concatenate([xTb8, xTq8], axis=1)),
            "cpk": _bf(np.concatenate([w2t, mask, identb], axis=1)),
            "wka": _bf(hpack(wk[:, hd0:hd0 + DPC], DPC)),
            "wqa": _bf(hpack(wq_s[:, hd0:hd0 + DPC], DPC)),
            "wva": _bf(hpack(wv[:, hd0:hd0 + DPC], DPC)),
            "woa": _bf(np.ascontiguousarray(
                wo[hd0:hd0 + DPC].reshape(4, 128, D)
                .transpose(1, 0, 2).reshape(128, 4 * D))),
            "b1c": b1c, "b2h": b2h,
        })
    return in_maps


def run(inputs: dict, trace: bool = False):
    """Build, run on 8 cores, return (full_output, BassKernelResults)."""
    nc = build_program()
    in_maps = _host_inputs(**inputs)
    res = run_bass_kernel_spmd(nc, in_maps, core_ids=list(range(N_CORES)),
                               trace=trace)
    out = np.zeros((B, L, D), np.float32)
    for b in range(B):
        out[b] = res.results[2 * b]["out"] + res.results[2 * b + 1]["out"]
    return out, res


def kernel(**inputs) -> np.ndarray:
    out, _ = run(inputs, trace=False)
    return out


# revision 22
# speedup vs baseline: 1.1592x; 1.0189x over previous
"""Trainium2 Bass kernel for CausalModulatedAttention.

Full-input contract: kernel(**inputs) takes the unsharded numpy inputs and
returns the full (B, L, D) float32 output.

Sharding: core = 2*b + g (b = batch, g = head-group).  The two cores of a
batch split the 16 heads (8 each) but both cover all 512 rows, with TRUE
causal extents per 128-row chunk (jext = 128*(ic+1)) -- no wasted score
columns.  The pairwise causal-graph bias G (shared by all heads) is instead
row-sharded across the pair -- core g computes G rows {0,3} or {1,2} chunks
(widths 256/512, SPMD-uniform) -- and the 192KB bias tiles are exchanged
through a per-pair AllGather.  Each core produces a partial output (its
heads' half of the d-contraction in the final projection); the host adds
the two halves.

Per core:
  - k/q/v projections in transposed layouts straight from x^T (PE),
    head-group slices only
  - pairwise G: gelu(he[j,c]+hc[i,c]+b1[c]) as one ACT op per 4-row group
    (per-partition bias), reduced over c via per-t stationary matrices (PE)
  - scores = q.k^T (PE); bias+mask tile added into PSUM on DVE
  - softmax without max-subtraction; Exp emits row sums via accum_out
  - transpose+normalize fused: EnT = E_chunk^T @ diag(1/rowsum) on PE
  - attn @ v on PE (col-group packed head pairs); partial output proj (PE)
All matmul operands bf16, fp32 PSUM accumulation.
"""

import math

import numpy as np
import ml_dtypes

import concourse.bass as bass
import concourse.mybir as mybir
import concourse.tile as tile
from concourse import bacc
from concourse.bass_utils import run_bass_kernel_spmd

BF = mybir.dt.bfloat16
F32 = mybir.dt.float32
AF = mybir.ActivationFunctionType
ALU = mybir.AluOpType

B, L, D = 4, 512, 1024
H, HD, CD = 16, 64, 32
ALPHA = 0.3
N_CORES = 8
HPC = 8               # heads per core
DPC = HPC * HD        # 512 d-columns per core
GJX = [256, 512]      # pairwise G width for (lo, hi) owned row chunk
NEG = -1.0e30
GW = GJX[0] + GJX[1]  # 768: packed G width per core
# packed bf16 consts: w2t (4096) | mask (768) | ident (128)
CPK = 4096 + GW + 128


def _bf(a):
    return np.ascontiguousarray(a.astype(ml_dtypes.bfloat16))


def _f32(a):
    return np.ascontiguousarray(a.astype(np.float32))


def core_rows(g):
    """Global row ranges (lo, hi) whose G rows core-group g computes."""
    lo = range(g * 128, g * 128 + 128)
    hi = range(384 - g * 128, 384 - g * 128 + 128)
    return lo, hi


def build_program():
    nc = bacc.Bacc("TRN2", num_devices=N_CORES, target_bir_lowering=False,
                   debug=False)

    boot_d = nc.dram_tensor("boot", [128, 1280], BF, kind="ExternalInput")
    xall_d = nc.dram_tensor("xall", [128, 8 * L + 8 * 256], BF, kind="ExternalInput")
    cpk_d = nc.dram_tensor("cpk", [128, CPK], BF, kind="ExternalInput")
    wk_d = nc.dram_tensor("wka", [128, 8 * DPC], BF, kind="ExternalInput")
    wq_d = nc.dram_tensor("wqa", [128, 8 * DPC], BF, kind="ExternalInput")
    wv_d = nc.dram_tensor("wva", [128, 8 * DPC], BF, kind="ExternalInput")
    wo_d = nc.dram_tensor("woa", [128, 4 * D], BF, kind="ExternalInput")
    b1_d = nc.dram_tensor("b1c", [CD, 1], F32, kind="ExternalInput")
    b2_d = nc.dram_tensor("b2h", [128, 1], F32, kind="ExternalInput")
    out_d = nc.dram_tensor("out", [L, D], F32, kind="ExternalOutput")

    with tile.TileContext(nc) as tc:
        with (
            tc.tile_pool(name="consts", bufs=1) as consts,
            tc.tile_pool(name="work", bufs=3) as work,
            tc.tile_pool(name="entp", bufs=6) as entp,
            tc.tile_pool(name="dram", bufs=1, space="DRAM") as dpool,
            tc.tile_pool(name="ppbig", bufs=3, space="PSUM") as ppbig,
            tc.tile_pool(name="ppg", bufs=1, space="PSUM") as ppg,
            tc.tile_pool(name="pptp", bufs=2, space="PSUM") as pptp,
            tc.tile_pool(name="ppot", bufs=2, space="PSUM") as ppot,
        ):
            def load(name, shape, dt, src):
                t = consts.tile(shape, dt, tag=name)
                nc.sync.dma_start(out=t[:], in_=src)
                return t

            boot = load("boot", [128, 1280], BF, boot_d[:, :])
            xall = load("xall", [128, 8 * L + 8 * 256], BF, xall_d[:, :])
            cpk = load("cpk", [128, CPK], BF, cpk_d[:, :])
            b1c = load("b1c", [CD, 1], F32, b1_d[:, :])
            b2h = load("b2h", [128, 1], F32, b2_d[:, :])
            wka = load("wka", [128, 8 * DPC], BF, wk_d[:, :])
            wqa = load("wqa", [128, 8 * DPC], BF, wq_d[:, :])
            wva = load("wva", [128, 8 * DPC], BF, wv_d[:, :])
            woa = load("woa", [128, 4 * D], BF, wo_d[:, :])

            xT = [xall[:, mc * L:(mc + 1) * L] for mc in range(8)]
            xTq = [xall[:, 8 * L + mc * 256: 8 * L + (mc + 1) * 256]
                   for mc in range(8)]
            wc1 = boot[:, 0:256]
            we1x4 = boot[:, 256:1280]
            w2t = cpk[:, 0:4096]
            maskc = cpk[:, 4096:4096 + GW]
            ident = cpk[:, 4096 + GW:4096 + GW + 128]
            wk = [wka[:, mc * DPC:(mc + 1) * DPC] for mc in range(8)]
            wq = [wqa[:, mc * DPC:(mc + 1) * DPC] for mc in range(8)]
            wv = [wva[:, mc * DPC:(mc + 1) * DPC] for mc in range(8)]
            wo = [woa[:, dc * D:(dc + 1) * D] for dc in range(4)]

            # ---------- he / hc (unblock the gelu chain) ----------
            ps = ppbig.tile([128, L], F32, tag="ps")
            for mc in range(8):
                nc.tensor.matmul(ps[:], we1x4[:, mc * 128:(mc + 1) * 128], xT[mc],
                                 start=(mc == 0), stop=(mc == 7))
            he4 = consts.tile([128, L], BF, tag="he4")
            nc.scalar.copy(he4[:], ps[:])

            ps = ppbig.tile([CD, 256], F32, tag="ps")
            for mc in range(8):
                nc.tensor.matmul(ps[:], wc1[:, mc * CD:(mc + 1) * CD], xTq[mc],
                                 start=(mc == 0), stop=(mc == 7))
            hcbT = consts.tile([CD, 256], F32, tag="hcbT")
            nc.vector.tensor_scalar_add(hcbT[:], ps[:], b1c[:, 0:1])
            hc4 = consts.tile([128, 64], F32, tag="hc4")
            hsrc = hcbT[:, :].rearrange("p (a t f) -> p a t f", a=2, t=32)
            for u in range(4):
                nc.scalar.dma_start(
                    out=hc4[u * CD:(u + 1) * CD, :].rearrange("p (a t) -> p a t", a=2),
                    in_=hsrc[:, :, :, u])

            # ---------- pairwise causal-graph bias (owned rows) ----------
            gsend = consts.tile([128, GW], BF, tag="gsend")

            def pairwise(oc):           # oc: owned chunk 0 (lo) / 1 (hi)
                jx = GJX[oc]
                moff = 0 if oc == 0 else GJX[0]
                graw = ppg.tile([128, 512], F32, tag="graw")
                for t in range(32):
                    # true causal width for this 4-row group, rounded up to
                    # cover both row-groups' SPMD-shared shape; columns
                    # beyond fd land under the -inf mask.  t==0 spans the
                    # full width so start=True clears has_written everywhere.
                    fd = jx if t == 0 else min(jx, (jx - 128) + 4 * t + 4)
                    ga = work.tile([128, fd], BF, tag=f"ga{oc}")
                    nc.scalar.activation(ga[:], he4[:, :fd], AF.Gelu,
                                         bias=hc4[:, oc * 32 + t: oc * 32 + t + 1])
                    nc.tensor.matmul(graw[:, :fd], w2t[:, t * 128:(t + 1) * 128],
                                     ga[:], start=(t == 0), stop=(t == 31))
                th = work.tile([128, jx], BF, tag=f"th{oc}")
                nc.scalar.activation(th[:], graw[:, :jx], AF.Tanh, scale=0.5,
                                     bias=b2h[:, 0:1])
                nc.vector.scalar_tensor_tensor(
                    gsend[:, moff:moff + jx], th[:], ALPHA / 2.0,
                    maskc[:, moff:moff + jx], op0=ALU.mult, op1=ALU.add)

            # two-phase exchange: hi chunks first so the wide attention
            # work unblocks while the lo gelus still run
            gmap = {}

            def exchange(oc):
                jx = GJX[oc]
                moff = 0 if oc == 0 else GJX[0]
                gin = dpool.tile([128, jx], BF, tag=f"gin{oc}")
                gout = dpool.tile([2, 128, jx], BF, tag=f"gout{oc}")
                nc.scalar.dma_start(out=gin[:], in_=gsend[:, moff:moff + jx])
                nc.gpsimd.collective_compute(
                    "AllGather", ALU.bypass,
                    replica_groups=[[0, 1], [2, 3], [4, 5], [6, 7]],
                    ins=[gin[:, :].opt()], outs=[gout[:, :, :].opt()])
                ga_ = consts.tile([128, jx], BF, tag=f"gx{oc}0")
                gb_ = consts.tile([128, jx], BF, tag=f"gx{oc}1")
                nc.scalar.dma_start(out=ga_[:], in_=gout[0, :, :])
                nc.scalar.dma_start(out=gb_[:], in_=gout[1, :, :])
                # G rows {0,3} came from rank 0, {1,2} from rank 1
                if oc == 0:
                    gmap[0], gmap[1] = ga_, gb_
                else:
                    gmap[3], gmap[2] = ga_, gb_

            pairwise(1)
            exchange(1)
            pairwise(0)
            exchange(0)

            # ---------- projection emitters ----------
            kT, qT, v = [None] * 4, [None] * 4, [None] * 4

            def proj_kq(dc):
                ps = ppbig.tile([128, L], F32, tag="ps")
                for mc in range(8):
                    nc.tensor.matmul(ps[:], wk[mc][:, dc * 128:(dc + 1) * 128],
                                     xT[mc], start=(mc == 0), stop=(mc == 7))
                t = consts.tile([128, L], BF, tag=f"kT{dc}")
                nc.vector.tensor_copy(t[:], ps[:])
                kT[dc] = t
                ps = ppbig.tile([128, L], F32, tag="ps")
                for mc in range(8):
                    nc.tensor.matmul(ps[:], wq[mc][:, dc * 128:(dc + 1) * 128],
                                     xT[mc], start=(mc == 0), stop=(mc == 7))
                t = consts.tile([128, L], BF, tag=f"qT{dc}")
                nc.vector.tensor_copy(t[:], ps[:])
                qT[dc] = t

            def proj_v(jc):
                t = consts.tile([128, DPC], BF, tag=f"v{jc}")
                ps = ppbig.tile([128, DPC], F32, tag="ps")
                for mc in range(8):
                    nc.tensor.matmul(ps[:], xT[mc][:, jc * 128:(jc + 1) * 128],
                                     wv[mc], start=(mc == 0), stop=(mc == 7))
                nc.vector.tensor_copy(t[:], ps[:])
                v[jc] = t

            # ---------- attention ----------
            ot = [[None] * 4 for _ in range(4)]

            def attention(ic, hp):
                jx = 128 * (ic + 1)
                njc = ic + 1
                gt = gmap[ic]
                otp = ppot.tile([128, 128], F32, tag="otp")
                for sub in range(2):
                    h = 2 * hp + sub
                    po = 64 * sub
                    sc = ppbig.tile([128, 512], F32, tag="ps")
                    nc.tensor.matmul(
                        sc[:, :jx], qT[hp][po:po + 64, ic * 128:(ic + 1) * 128],
                        kT[hp][po:po + 64, :jx], start=True, stop=True,
                        tile_position=(po, 0))
                    nc.vector.tensor_add(sc[:, :jx], sc[:, :jx], gt[:, :jx])
                    e = work.tile([128, jx], BF, tag=f"e{ic}")
                    sums = work.tile([128, 1], F32, tag="sums")
                    nc.scalar.activation(e[:], sc[:, :jx], AF.Exp,
                                         accum_out=sums[:, 0:1])
                    inv = work.tile([128, 1], F32, tag="inv")
                    nc.vector.reciprocal(inv[:], sums[:])
                    dg = work.tile([128, 128], BF, tag="dg")
                    nc.vector.tensor_scalar_mul(dg[:], ident, inv[:, 0:1])
                    for jc in range(njc):
                        etp = pptp.tile([128, 128], F32, tag="etp")
                        nc.tensor.matmul(etp[:], e[:, jc * 128:(jc + 1) * 128],
                                         dg[:], start=True, stop=True)
                        ent = entp.tile([128, 128], BF, tag="ent")
                        if jc % 2 == 0:
                            nc.vector.tensor_copy(ent[:], etp[:])
                        else:
                            nc.scalar.copy(ent[:], etp[:])
                        nc.tensor.matmul(
                            otp[po:po + 64, :], v[jc][:, h * HD:(h + 1) * HD],
                            ent[:], start=(jc == 0), stop=(jc == njc - 1),
                            tile_position=(0, po))
                t = consts.tile([128, 128], BF, tag=f"ot{ic}_{hp}")
                nc.vector.tensor_copy(t[:], otp[:])
                ot[ic][hp] = t

            def out_proj(ic, nn):
                ps = ppbig.tile([128, 512], F32, tag="ps")
                for dc in range(4):
                    nc.tensor.matmul(ps[:], ot[ic][dc][:],
                                     wo[dc][:, nn * 512:(nn + 1) * 512],
                                     start=(dc == 0), stop=(dc == 3))
                osb = work.tile([128, 512], F32, tag="osb")
                nc.vector.tensor_copy(osb[:], ps[:])
                nc.sync.dma_start(
                    out=out_d[ic * 128:(ic + 1) * 128, nn * 512:(nn + 1) * 512],
                    in_=osb[:])

            # emission: projections, then attention largest-first with the
            # finished chunks' output projections interleaved as PE fillers
            for dc in range(4):
                proj_kq(dc)
                proj_v(dc)
            for hp in range(4):
                attention(3, hp)
            for hp in range(4):
                attention(2, hp)
                if hp >= 2:
                    out_proj(3, hp - 2)
            for hp in range(4):
                attention(1, hp)
                if hp >= 2:
                    out_proj(2, hp - 2)
            for hp in range(4):
                attention(0, hp)
                if hp >= 2:
                    out_proj(1, hp - 2)
            out_proj(0, 0)
            out_proj(0, 1)

    nc.compile()
    return nc


def _host_inputs(x, Wq, Wk, Wv, Wo, Wc, We, W1c, W1e, b1, W2, b2):
    """Per-core input dicts (host-side shard/cast/pack)."""
    x = _f32(np.asarray(x))
    wq_s = _f32(np.asarray(Wq) / math.sqrt(HD))
    wk = _f32(np.asarray(Wk))
    wv = _f32(np.asarray(Wv))
    wo = _f32(np.asarray(Wo))
    wc1 = _f32(np.asarray(Wc) @ np.asarray(W1c))      # (D, CD)
    we1 = _f32(np.asarray(We) @ np.asarray(W1e))
    wc1r = wc1.reshape(8, 128, CD).transpose(1, 0, 2).reshape(128, 8 * CD)
    we1c = we1.reshape(8, 128, CD).transpose(1, 0, 2)          # (128, 8, CD)
    we1x4 = np.tile(we1c[:, :, None, :], (1, 1, 4, 1)).reshape(128, 8 * 128)
    b1c = _f32(np.asarray(b1).reshape(CD, 1))
    b2h = _f32(np.full((128, 1), 0.5 * float(np.asarray(b2).reshape(-1)[0])))
    w2 = _f32(np.asarray(W2))

    # w2t[p=u*32+c, t*128 + m] = W2[c] if m == 4t+u else 0
    w2t = np.zeros((32, 128, 128), np.float32)
    for t in range(32):
        for u in range(4):
            w2t[t, u * CD:(u + 1) * CD, 4 * t + u] = w2
    w2t = w2t.transpose(1, 0, 2).reshape(128, 32 * 128)

    identb = np.eye(128, dtype=np.float32)
    bootc = np.concatenate([wc1r, we1x4], axis=1)

    def hpack(w, cols):  # (1024, cols) -> (128, 8*cols) m-chunk-major
        return w.reshape(8, 128, cols).transpose(1, 0, 2).reshape(128, 8 * cols)

    in_maps = []
    for core in range(N_CORES):
        b, g = core // 2, core % 2
        lo, hi = core_rows(g)
        rows = np.concatenate([np.arange(lo.start, lo.stop),
                               np.arange(hi.start, hi.stop)])
        hd0 = g * DPC                                  # head-group d offset
        xTb = np.ascontiguousarray(x[b].T)             # (D, L)
        mask = np.zeros((128, GW), np.float32)
        moff = 0
        for oc, rng in enumerate((lo, hi)):
            jx = GJX[oc]
            jj = np.arange(jx)[None, :]
            rr = np.arange(rng.start, rng.stop)[:, None]
            mask[:, moff:moff + jx] = np.where(jj <= rr, 0.0, NEG)
            moff += jx
        xTb8 = hpack(xTb, L)
        xTq8 = hpack(np.ascontiguousarray(xTb[:, rows]), 256)
        in_maps.append({
            "boot": _bf(bootc),
            "xall": _bf(np.concatenate([xTb8, xTq8], axis=1)),
            "cpk": _bf(np.concatenate([w2t, mask, identb], axis=1)),
            "wka": _bf(hpack(wk[:, hd0:hd0 + DPC], DPC)),
            "wqa": _bf(hpack(wq_s[:, hd0:hd0 + DPC], DPC)),
            "wva": _bf(hpack(wv[:, hd0:hd0 + DPC], DPC)),
            "woa": _bf(np.ascontiguousarray(
                wo[hd0:hd0 + DPC].reshape(4, 128, D)
                .transpose(1, 0, 2).reshape(128, 4 * D))),
            "b1c": b1c, "b2h": b2h,
        })
    return in_maps


def run(inputs: dict, trace: bool = False):
    """Build, run on 8 cores, return (full_output, BassKernelResults)."""
    nc = build_program()
    in_maps = _host_inputs(**inputs)
    res = run_bass_kernel_spmd(nc, in_maps, core_ids=list(range(N_CORES)),
                               trace=trace)
    out = np.zeros((B, L, D), np.float32)
    for b in range(B):
        out[b] = res.results[2 * b]["out"] + res.results[2 * b + 1]["out"]
    return out, res


def kernel(**inputs) -> np.ndarray:
    out, _ = run(inputs, trace=False)
    return out


# revision 24
# speedup vs baseline: 1.5051x; 1.2984x over previous
"""Trainium2 Bass kernel for CausalModulatedAttention.

Full-input contract: kernel(**inputs) takes the unsharded numpy inputs and
returns the full (B, L, D) float32 output.

Sharding: core = 2*b + g (b = batch, g = head-group).  The two cores of a
batch split the 16 heads (8 each) but both cover all 512 rows, with TRUE
causal extents per 128-row chunk (jext = 128*(ic+1)) -- no wasted score
columns.  The pairwise causal-graph bias G (shared by all heads) is instead
row-sharded across the pair -- core g computes G rows {0,3} or {1,2} chunks
(widths 256/512, SPMD-uniform) -- and the 192KB bias tiles are exchanged
through a per-pair AllGather.  Each core produces a partial output (its
heads' half of the d-contraction in the final projection); the host adds
the two halves.

Per core:
  - k/q/v projections in transposed layouts straight from x^T (PE),
    head-group slices only
  - pairwise G: gelu(he[j,c]+hc[i,c]+b1[c]) as one ACT op per 4-row group
    (per-partition bias), reduced over c via per-t stationary matrices (PE)
  - scores = q.k^T (PE); bias+mask tile added into PSUM on DVE
  - softmax without max-subtraction; Exp emits row sums via accum_out
  - transpose+normalize fused: EnT = E_chunk^T @ diag(1/rowsum) on PE
  - attn @ v on PE (col-group packed head pairs); partial output proj (PE)
All matmul operands bf16, fp32 PSUM accumulation.
"""

import math

import numpy as np
import ml_dtypes

import concourse.bass as bass
import concourse.mybir as mybir
import concourse.tile as tile
from concourse import bacc
from concourse.bass_utils import run_bass_kernel_spmd

BF = mybir.dt.bfloat16
F32 = mybir.dt.float32
AF = mybir.ActivationFunctionType
ALU = mybir.AluOpType

B, L, D = 4, 512, 1024
H, HD, CD = 16, 64, 32
ALPHA = 0.3
N_CORES = 8
HPC = 8               # heads per core
DPC = HPC * HD        # 512 d-columns per core
GJX = [256, 512]      # pairwise G width for (lo, hi) owned row chunk
NEG = -1.0e30
GW = GJX[0] + GJX[1]  # 768: packed G width per core
# packed bf16 consts: w2t (4096) | mask (768) | ident (128)
CPK = 4096 + GW + 128


def _bf(a):
    return np.ascontiguousarray(a.astype(ml_dtypes.bfloat16))


def _f32(a):
    return np.ascontiguousarray(a.astype(np.float32))


def core_rows(g):
    """Global row ranges (lo, hi) whose G rows core-group g computes."""
    lo = range(g * 128, g * 128 + 128)
    hi = range(384 - g * 128, 384 - g * 128 + 128)
    return lo, hi


def build_program():
    nc = bacc.Bacc("TRN2", num_devices=N_CORES, target_bir_lowering=False,
                   debug=False)

    boot_d = nc.dram_tensor("boot", [128, 1280], BF, kind="ExternalInput")
    xall_d = nc.dram_tensor("xall", [128, 8 * L + 8 * 256], BF, kind="ExternalInput")
    cpk_d = nc.dram_tensor("cpk", [128, CPK], BF, kind="ExternalInput")
    wk_d = nc.dram_tensor("wka", [128, 8 * DPC], BF, kind="ExternalInput")
    wq_d = nc.dram_tensor("wqa", [128, 8 * DPC], BF, kind="ExternalInput")
    wv_d = nc.dram_tensor("wva", [128, 8 * DPC], BF, kind="ExternalInput")
    wo_d = nc.dram_tensor("woa", [128, 4 * D], BF, kind="ExternalInput")
    b1_d = nc.dram_tensor("b1x4", [128, 1], F32, kind="ExternalInput")
    b2_d = nc.dram_tensor("b2h", [128, 1], F32, kind="ExternalInput")
    out_d = nc.dram_tensor("out", [L, D], F32, kind="ExternalOutput")

    with tile.TileContext(nc) as tc:
        with (
            tc.tile_pool(name="consts", bufs=1) as consts,
            tc.tile_pool(name="work", bufs=3) as work,
            tc.tile_pool(name="entp", bufs=6) as entp,
            tc.tile_pool(name="dram", bufs=1, space="DRAM") as dpool,
            tc.tile_pool(name="ppbig", bufs=4, space="PSUM") as ppbig,
            tc.tile_pool(name="pptp", bufs=2, space="PSUM") as pptp,
            tc.tile_pool(name="ppot", bufs=2, space="PSUM") as ppot,
        ):
            def load(name, shape, dt, src):
                t = consts.tile(shape, dt, tag=name)
                nc.sync.dma_start(out=t[:], in_=src)
                return t

            boot = load("boot", [128, 1280], BF, boot_d[:, :])
            xall = load("xall", [128, 8 * L + 8 * 256], BF, xall_d[:, :])
            cpk = load("cpk", [128, CPK], BF, cpk_d[:, :])
            b1x4 = load("b1x4", [128, 1], F32, b1_d[:, :])
            b2h = load("b2h", [128, 1], F32, b2_d[:, :])
            wka = load("wka", [128, 8 * DPC], BF, wk_d[:, :])
            wqa = load("wqa", [128, 8 * DPC], BF, wq_d[:, :])
            wva = load("wva", [128, 8 * DPC], BF, wv_d[:, :])
            woa = load("woa", [128, 4 * D], BF, wo_d[:, :])

            xT = [xall[:, mc * L:(mc + 1) * L] for mc in range(8)]
            xTq = [xall[:, 8 * L + mc * 256: 8 * L + (mc + 1) * 256]
                   for mc in range(8)]
            wc1 = boot[:, 0:256]
            we1x4 = boot[:, 256:1280]
            w2t = cpk[:, 0:4096]
            maskc = cpk[:, 4096:4096 + GW]
            ident = cpk[:, 4096 + GW:4096 + GW + 128]
            wk = [wka[:, mc * DPC:(mc + 1) * DPC] for mc in range(8)]
            wq = [wqa[:, mc * DPC:(mc + 1) * DPC] for mc in range(8)]
            wv = [wva[:, mc * DPC:(mc + 1) * DPC] for mc in range(8)]
            wo = [woa[:, dc * D:(dc + 1) * D] for dc in range(4)]

            # warm up the CC channel so the real exchanges are fast
            di = dpool.tile([128, 16], BF, tag="di")
            do = dpool.tile([2, 128, 16], BF, tag="do")
            dsb = consts.tile([128, 16], BF, tag="dsb")
            nc.vector.memset(dsb[:], 0.0)
            nc.gpsimd.dma_start(out=di[:], in_=dsb[:])
            nc.gpsimd.collective_compute(
                "AllGather", ALU.bypass,
                replica_groups=[[0, 1], [2, 3], [4, 5], [6, 7]],
                ins=[di[:, :].opt()], outs=[do[:, :, :].opt()])

            # ---------- he / hc (unblock the gelu chain) ----------
            ps = ppbig.tile([128, L], F32, tag="ps")
            for mc in range(8):
                nc.tensor.matmul(ps[:], we1x4[:, mc * 128:(mc + 1) * 128], xT[mc],
                                 start=(mc == 0), stop=(mc == 7))
            he4 = consts.tile([128, L], BF, tag="he4")
            nc.scalar.copy(he4[:], ps[:])

            # hc4[u*32+c, oc*32+t] = (x @ Wc1)[oc*128+4t+u, c] + b1[c]
            # built directly on PE with a column-strided moving operand
            ps = ppbig.tile([128, 64], F32, tag="ps")
            for u in range(4):
                for mc in range(8):
                    rsrc = xTq[mc].rearrange("p (a t f) -> p a t f", a=2, f=4)[:, :, :, u]
                    nc.tensor.matmul(ps[u * CD:(u + 1) * CD, :],
                                     wc1[:, mc * CD:(mc + 1) * CD], rsrc,
                                     start=(mc == 0), stop=(mc == 7),
                                     tile_position=(0, u * CD))
            hc4 = consts.tile([128, 64], F32, tag="hc4")
            nc.vector.tensor_scalar_add(hc4[:], ps[:], b1x4[:, 0:1])

            # ---------- pairwise causal-graph bias (owned rows) ----------
            gsend = consts.tile([128, GW], BF, tag="gsend")

            def pairwise(oc):           # oc: owned chunk 0 (lo) / 1 (hi)
                jx = GJX[oc]
                moff = 0 if oc == 0 else GJX[0]
                graw = ppbig.tile([128, 512], F32, tag="ps")
                for t in range(32):
                    # true causal width for this 4-row group, rounded up to
                    # cover both row-groups' SPMD-shared shape; columns
                    # beyond fd land under the -inf mask.  t==0 spans the
                    # full width so start=True clears has_written everywhere.
                    fd = jx if t == 0 else min(jx, (jx - 128) + 4 * t + 4)
                    ga = work.tile([128, fd], BF, tag=f"ga{oc}")
                    nc.scalar.activation(ga[:], he4[:, :fd], AF.Gelu,
                                         bias=hc4[:, oc * 32 + t: oc * 32 + t + 1])
                    nc.tensor.matmul(graw[:, :fd], w2t[:, t * 128:(t + 1) * 128],
                                     ga[:], start=(t == 0), stop=(t == 31))
                th = work.tile([128, jx], BF, tag=f"th{oc}")
                nc.scalar.activation(th[:], graw[:, :jx], AF.Tanh, scale=0.5,
                                     bias=b2h[:, 0:1])
                nc.vector.scalar_tensor_tensor(
                    gsend[:, moff:moff + jx], th[:], ALPHA / 2.0,
                    maskc[:, moff:moff + jx], op0=ALU.mult, op1=ALU.add)

            # two-phase exchange: hi chunks first so the wide attention
            # work unblocks while the lo gelus still run
            gmap = {}

            def exchange(oc):
                jx = GJX[oc]
                moff = 0 if oc == 0 else GJX[0]
                gin = dpool.tile([128, jx], BF, tag=f"gin{oc}")
                gout = dpool.tile([2, 128, jx], BF, tag=f"gout{oc}")
                nc.scalar.dma_start(out=gin[:], in_=gsend[:, moff:moff + jx])
                nc.gpsimd.collective_compute(
                    "AllGather", ALU.bypass,
                    replica_groups=[[0, 1], [2, 3], [4, 5], [6, 7]],
                    ins=[gin[:, :].opt()], outs=[gout[:, :, :].opt()])
                ga_ = consts.tile([128, jx], BF, tag=f"gx{oc}0")
                gb_ = consts.tile([128, jx], BF, tag=f"gx{oc}1")
                nc.scalar.dma_start(out=ga_[:], in_=gout[0, :, :])
                nc.scalar.dma_start(out=gb_[:], in_=gout[1, :, :])
                # G rows {0,3} came from rank 0, {1,2} from rank 1
                if oc == 0:
                    gmap[0], gmap[1] = ga_, gb_
                else:
                    gmap[3], gmap[2] = ga_, gb_

            pairwise(1)
            exchange(1)
            pairwise(0)
            exchange(0)

            # ---------- projection emitters ----------
            kT, qT, v = [None] * 4, [None] * 4, [None] * 4

            def proj_kq(dc):
                ps = ppbig.tile([128, L], F32, tag="ps")
                for mc in range(8):
                    nc.tensor.matmul(ps[:], wk[mc][:, dc * 128:(dc + 1) * 128],
                                     xT[mc], start=(mc == 0), stop=(mc == 7))
                t = consts.tile([128, L], BF, tag=f"kT{dc}")
                nc.vector.tensor_copy(t[:], ps[:])
                kT[dc] = t
                ps = ppbig.tile([128, L], F32, tag="ps")
                for mc in range(8):
                    nc.tensor.matmul(ps[:], wq[mc][:, dc * 128:(dc + 1) * 128],
                                     xT[mc], start=(mc == 0), stop=(mc == 7))
                t = consts.tile([128, L], BF, tag=f"qT{dc}")
                nc.vector.tensor_copy(t[:], ps[:])
                qT[dc] = t

            def proj_v(jc):
                t = consts.tile([128, DPC], BF, tag=f"v{jc}")
                ps = ppbig.tile([128, DPC], F32, tag="ps")
                for mc in range(8):
                    nc.tensor.matmul(ps[:], xT[mc][:, jc * 128:(jc + 1) * 128],
                                     wv[mc], start=(mc == 0), stop=(mc == 7))
                nc.vector.tensor_copy(t[:], ps[:])
                v[jc] = t

            # ---------- attention ----------
            ot = [[None] * 4 for _ in range(4)]

            def attention(ic, hp):
                jx = 128 * (ic + 1)
                njc = ic + 1
                gt = gmap[ic]
                otp = ppot.tile([128, 128], F32, tag="otp")
                for sub in range(2):
                    h = 2 * hp + sub
                    po = 64 * sub
                    sc = ppbig.tile([128, 512], F32, tag="ps")
                    nc.tensor.matmul(
                        sc[:, :jx], qT[hp][po:po + 64, ic * 128:(ic + 1) * 128],
                        kT[hp][po:po + 64, :jx], start=True, stop=True,
                        tile_position=(po, 0))
                    nc.vector.tensor_add(sc[:, :jx], sc[:, :jx], gt[:, :jx])
                    e = work.tile([128, jx], BF, tag=f"e{ic}")
                    sums = work.tile([128, 1], F32, tag="sums")
                    nc.scalar.activation(e[:], sc[:, :jx], AF.Exp,
                                         accum_out=sums[:, 0:1])
                    inv = work.tile([128, 1], F32, tag="inv")
                    nc.vector.reciprocal(inv[:], sums[:])
                    dg = work.tile([128, 128], BF, tag="dg")
                    nc.vector.tensor_scalar_mul(dg[:], ident, inv[:, 0:1])
                    for jc in range(njc):
                        etp = pptp.tile([128, 128], F32, tag="etp")
                        nc.tensor.matmul(etp[:], e[:, jc * 128:(jc + 1) * 128],
                                         dg[:], start=True, stop=True)
                        ent = entp.tile([128, 128], BF, tag="ent")
                        if jc % 2 == 0:
                            nc.vector.tensor_copy(ent[:], etp[:])
                        else:
                            nc.scalar.copy(ent[:], etp[:])
                        nc.tensor.matmul(
                            otp[po:po + 64, :], v[jc][:, h * HD:(h + 1) * HD],
                            ent[:], start=(jc == 0), stop=(jc == njc - 1),
                            tile_position=(0, po))
                t = consts.tile([128, 128], BF, tag=f"ot{ic}_{hp}")
                nc.vector.tensor_copy(t[:], otp[:])
                ot[ic][hp] = t

            def out_proj(ic, nn):
                ps = ppbig.tile([128, 512], F32, tag="ps")
                for dc in range(4):
                    nc.tensor.matmul(ps[:], ot[ic][dc][:],
                                     wo[dc][:, nn * 512:(nn + 1) * 512],
                                     start=(dc == 0), stop=(dc == 3))
                osb = work.tile([128, 512], F32, tag="osb")
                nc.vector.tensor_copy(osb[:], ps[:])
                nc.sync.dma_start(
                    out=out_d[ic * 128:(ic + 1) * 128, nn * 512:(nn + 1) * 512],
                    in_=osb[:])

            # emission: projections, then attention largest-first with the
            # finished chunks' output projections interleaved as PE fillers
            for dc in range(4):
                proj_kq(dc)
                proj_v(dc)
            for hp in range(4):
                attention(3, hp)
            for hp in range(4):
                attention(2, hp)
                if hp >= 2:
                    out_proj(3, hp - 2)
            for hp in range(4):
                attention(1, hp)
                if hp >= 2:
                    out_proj(2, hp - 2)
            for hp in range(4):
                attention(0, hp)
                if hp >= 2:
                    out_proj(1, hp - 2)
            out_proj(0, 0)
            out_proj(0, 1)

    nc.compile()
    return nc


def _host_inputs(x, Wq, Wk, Wv, Wo, Wc, We, W1c, W1e, b1, W2, b2):
    """Per-core input dicts (host-side shard/cast/pack)."""
    x = _f32(np.asarray(x))
    wq_s = _f32(np.asarray(Wq) / math.sqrt(HD))
    wk = _f32(np.asarray(Wk))
    wv = _f32(np.asarray(Wv))
    wo = _f32(np.asarray(Wo))
    wc1 = _f32(np.asarray(Wc) @ np.asarray(W1c))      # (D, CD)
    we1 = _f32(np.asarray(We) @ np.asarray(W1e))
    wc1r = wc1.reshape(8, 128, CD).transpose(1, 0, 2).reshape(128, 8 * CD)
    we1c = we1.reshape(8, 128, CD).transpose(1, 0, 2)          # (128, 8, CD)
    we1x4 = np.tile(we1c[:, :, None, :], (1, 1, 4, 1)).reshape(128, 8 * 128)
    b1x4 = _f32(np.tile(np.asarray(b1).reshape(1, CD), (4, 1)).reshape(128, 1))
    b2h = _f32(np.full((128, 1), 0.5 * float(np.asarray(b2).reshape(-1)[0])))
    w2 = _f32(np.asarray(W2))

    # w2t[p=u*32+c, t*128 + m] = W2[c] if m == 4t+u else 0
    w2t = np.zeros((32, 128, 128), np.float32)
    for t in range(32):
        for u in range(4):
            w2t[t, u * CD:(u + 1) * CD, 4 * t + u] = w2
    w2t = w2t.transpose(1, 0, 2).reshape(128, 32 * 128)

    identb = np.eye(128, dtype=np.float32)
    bootc = np.concatenate([wc1r, we1x4], axis=1)

    def hpack(w, cols):  # (1024, cols) -> (128, 8*cols) m-chunk-major
        return w.reshape(8, 128, cols).transpose(1, 0, 2).reshape(128, 8 * cols)

    in_maps = []
    for core in range(N_CORES):
        b, g = core // 2, core % 2
        lo, hi = core_rows(g)
        rows = np.concatenate([np.arange(lo.start, lo.stop),
                               np.arange(hi.start, hi.stop)])
        hd0 = g * DPC                                  # head-group d offset
        xTb = np.ascontiguousarray(x[b].T)             # (D, L)
        mask = np.zeros((128, GW), np.float32)
        moff = 0
        for oc, rng in enumerate((lo, hi)):
            jx = GJX[oc]
            jj = np.arange(jx)[None, :]
            rr = np.arange(rng.start, rng.stop)[:, None]
            mask[:, moff:moff + jx] = np.where(jj <= rr, 0.0, NEG)
            moff += jx
        xTb8 = hpack(xTb, L)
        xTq8 = hpack(np.ascontiguousarray(xTb[:, rows]), 256)
        in_maps.append({
            "boot": _bf(bootc),
            "xall": _bf(np.concatenate([xTb8, xTq8], axis=1)),
            "cpk": _bf(np.concatenate([w2t, mask, identb], axis=1)),
            "wka": _bf(hpack(wk[:, hd0:hd0 + DPC], DPC)),
            "wqa": _bf(hpack(wq_s[:, hd0:hd0 + DPC], DPC)),
            "wva": _bf(hpack(wv[:, hd0:hd0 + DPC], DPC)),
            "woa": _bf(np.ascontiguousarray(
                wo[hd0:hd0 + DPC].reshape(4, 128, D)
                .transpose(1, 0, 2).reshape(128, 4 * D))),
            "b1x4": b1x4, "b2h": b2h,
        })
    return in_maps


def run(inputs: dict, trace: bool = False):
    """Build, run on 8 cores, return (full_output, BassKernelResults)."""
    nc = build_program()
    in_maps = _host_inputs(**inputs)
    res = run_bass_kernel_spmd(nc, in_maps, core_ids=list(range(N_CORES)),
                               trace=trace)
    out = np.zeros((B, L, D), np.float32)
    for b in range(B):
        out[b] = res.results[2 * b]["out"] + res.results[2 * b + 1]["out"]
    return out, res


def kernel(**inputs) -> np.ndarray:
    out, _ = run(inputs, trace=False)
    return out
